# revision 30
# baseline (speedup 1.0000x reference)
"""Fused attention block (RMSNorm -> QKV -> 2D RoPE -> SDPA -> proj) on 8
Trainium2 NeuronCores, data-parallel over the batch dimension (B=8, one batch
element per core; no collectives).

Pipelined per-head-pair schedule: iteration i computes the QKV feature tiles
for pair i+1, the (row-tiled, 2-head-packed) score matmuls + softmax exp for
pair i, and the A@V matmuls for pair i-1, so the PE array and the scalar
(activation) engine stay busy concurrently.

Self-contained: hardcodes shapes B=8, N=1024, C=1024, H=16, D=64.
"""

import hashlib
import numpy as np
import ml_dtypes


def _salt_len():
    """Source-dependent dummy-input length: the remote AOT compile cache
    keys on the HLO signature and misses kernel-program changes, so vary
    the signature whenever this file changes."""
    try:
        with open(__file__, 'rb') as f:
            h = hashlib.sha256(f.read()).digest()
        return int.from_bytes(h[:2], 'little') + 1
    except OSError:
        return 1

B = 8
N = 1024
C = 1024
H = 16
D = 64
GRID = 32
EPS = 1e-6
P = 128
NT = N // P      # 8 token tiles
CT = C // P      # 8 channel tiles
PAIRS = H // 2   # 8 head pairs

BF16 = ml_dtypes.bfloat16


def _rope_tables():
    """Feature-major 2D-RoPE tables [N, 32] (cos/sin per rotation pair)."""
    quarter = D // 4
    freqs = 1.0 / (10000.0 ** (np.arange(quarter, dtype=np.float64) / quarter))
    ys, xs = np.meshgrid(np.arange(GRID, dtype=np.float64),
                         np.arange(GRID, dtype=np.float64), indexing='ij')
    ang_x = xs.reshape(-1)[:, None] * freqs[None, :]      # [N, 16]
    ang_y = ys.reshape(-1)[:, None] * freqs[None, :]      # [N, 16]
    angles = np.concatenate([ang_x, ang_y], axis=-1)      # [N, 32]
    return np.cos(angles), np.sin(angles)


def _rope_tables_rh():
    """Rotate-half-layout tables [128, N] for the permuted q/k dim order.

    Head dims are permuted to [evens, odds]; within each 64-dim head block:
      out[0:32]  = q[0:32]*cos - q[32:64]*sin
      out[32:64] = q[0:32]*sin + q[32:64]*cos
    so cosp = [cos, cos] and sinp = [-sin, +sin] per block, duplicated for
    the two heads sharing a feature tile.
    """
    cos, sin = _rope_tables()                             # [N, 32]
    blk_cos = np.concatenate([cos.T, cos.T], axis=0)      # [64, N]
    # sin sign keyed by SOURCE half (cross-term reads the opposite half, and
    # walrus requires both SBUF inputs of tensor_tensor to share a base
    # partition): reading odds (rows 32:64) contributes -sin, evens +sin.
    blk_sin = np.concatenate([sin.T, -sin.T], axis=0)     # [64, N]
    cosp = np.concatenate([blk_cos, blk_cos], axis=0)     # [128, N]
    sinp = np.concatenate([blk_sin, blk_sin], axis=0)
    return cosp.astype(np.float32), sinp.astype(np.float32)


def build(nc):
    import concourse.mybir as mybir
    import concourse.tile as tile
    from concourse.masks import make_identity

    f32 = mybir.dt.float32
    f16 = mybir.dt.float16
    bf16 = mybir.dt.bfloat16
    AF = mybir.ActivationFunctionType
    ALU = mybir.AluOpType

    x_ext = nc.declare_dram_parameter("x", [N, C], bf16, isOutput=False)
    wqkv_ext = nc.declare_dram_parameter("wqkv", [C, 3 * C], bf16, isOutput=False)
    wproj_ext = nc.declare_dram_parameter("wproj", [C, C], bf16, isOutput=False)
    bproj_ext = nc.declare_dram_parameter("bproj", [1, C], bf16, isOutput=False)
    cosp_ext = nc.declare_dram_parameter("cosp", [P, N], bf16, isOutput=False)
    sinp_ext = nc.declare_dram_parameter("sinp", [P, N], bf16, isOutput=False)
    out_ext = nc.declare_dram_parameter("out", [N, C], bf16, isOutput=True)
    nc.declare_dram_parameter("salt", [1, _salt_len()], f32, isOutput=False)

    from contextlib import ExitStack

    with tile.TileContext(nc) as tc:
        with ExitStack() as stack:
            ep = stack.enter_context
            consts = ep(tc.tile_pool(name="consts", bufs=1))
            w_pool = ep(tc.tile_pool(name="weights", bufs=1))
            tabs = ep(tc.tile_pool(name="tabs", bufs=1))
            xnt_pool = ep(tc.tile_pool(name="xnt", bufs=1))
            v_pool = ep(tc.tile_pool(name="vaug", bufs=1))
            o_pool = ep(tc.tile_pool(name="ofm", bufs=1))
            psB = ep(tc.tile_pool(name="psbig", bufs=2, space="PSUM"))
            psO = ep(tc.tile_pool(name="pso", bufs=2, space="PSUM"))
            qk_pool = ep(tc.tile_pool(name="qk", bufs=3))
            rt_pool = ep(tc.tile_pool(name="rope", bufs=2))
            et_pool = ep(tc.tile_pool(name="et", bufs=8))
            stg_pool = ep(tc.tile_pool(name="stg", bufs=2))
            r_pool = ep(tc.tile_pool(name="rstat", bufs=1))
            rb_pool = ep(tc.tile_pool(name="rbc", bufs=1))
            of_pool = ep(tc.tile_pool(name="outsb", bufs=2))
            ident = consts.tile([P, P], bf16, name="ident")
            make_identity(nc, ident[:])
            ones_col = consts.tile([1, P], bf16, name="ones_col")
            nc.vector.memset(ones_col[:], 1.0)
            eps_t = consts.tile([P, 1], f32, name="eps_t")
            nc.vector.memset(eps_t[:], EPS)
            bproj_sb = consts.tile([1, C], bf16, name="bproj_sb")
            nc.scalar.dma_start(bproj_sb[:], bproj_ext[:])
            cosp = tabs.tile([P, N], bf16, name="cosp")
            sinp = tabs.tile([P, N], bf16, name="sinp")
            nc.scalar.dma_start(cosp[:], cosp_ext[:])
            nc.scalar.dma_start(sinp[:], sinp_ext[:])

            # weights: wqkv per channel tile j; wproj one big tile
            wqkv_t = [w_pool.tile([P, 3 * C], bf16, name=f"wq{j}", tag=f"wq{j}")
                      for j in range(CT)]
            wproj_b = w_pool.tile([P, CT * C], bf16, name="wproj_b")
            # DMA order: q cols, k cols, v cols (matches first-use order),
            # wproj last on a separate queue.
            wproj_v = wproj_b.rearrange("p (j c) -> p j c", c=C)

            # xnt_big[:, j*N + n] = xn[n, j*128 + p] (normalized x, transposed)
            xnt_big = xnt_pool.tile([P, CT * N], bf16, name="xnt_big")
            xnt_v = xnt_big.rearrange("p (j n) -> p j n", n=N)
            # v token-major with ones column: [128, 16*65] per token tile
            v_aug = [v_pool.tile([P, H * (D + 1)], bf16, name=f"vaug{i}",
                                 tag=f"vaug{i}") for i in range(NT)]
            # attention output, feature-major: pair f at cols f*N..(f+1)*N
            o_big = o_pool.tile([P, PAIRS * N], bf16, name="o_big")
            o_v = o_big.rearrange("p (f n) -> p f n", n=N)

            # ---- phase A: norm + transpose --------------------------------
            with ExitStack() as astack:
                x_pool = astack.enter_context(tc.tile_pool(name="xin", bufs=8))
                st_pool = astack.enter_context(tc.tile_pool(name="stats", bufs=2))
                xn_pool = astack.enter_context(tc.tile_pool(name="xn", bufs=2))
                x_tiles = []
                for i in range(NT):
                    x_i = x_pool.tile([P, C], bf16, name="x_i")
                    x_tiles.append(x_i)
                for i in (0, 2, 1, 3, 5, 7):
                    eng = nc.sync if i % 2 == 0 else nc.scalar
                    eng.dma_start(x_tiles[i][:], x_ext[i * P:(i + 1) * P, :])
                # weight DMAs issue behind the ungated x tiles: q on sync
                # (scalar must stay free for the norm ACTs), then the
                # ring-gated x4/x6; k/v/wproj on gpsimd
                for j in range(CT):
                    nc.sync.dma_start(wqkv_t[j][:, 0:C],
                                      wqkv_ext[j * P:(j + 1) * P, 0:C])
                for i in (4, 6):
                    nc.sync.dma_start(x_tiles[i][:],
                                      x_ext[i * P:(i + 1) * P, :])
                for j in range(CT):
                    nc.gpsimd.dma_start(wqkv_t[j][:, 2 * C:],
                                        wqkv_ext[j * P:(j + 1) * P, 2 * C:])
                for j in range(CT):
                    nc.gpsimd.dma_start(wqkv_t[j][:, C:2 * C],
                                        wqkv_ext[j * P:(j + 1) * P, C:2 * C])
                for j in range(CT):
                    nc.gpsimd.dma_start(wproj_v[:, j, :],
                                        wproj_ext[j * P:(j + 1) * P, :])
                for i in range(NT):
                    x_i = x_tiles[i]
                    xn = xn_pool.tile([P, C], bf16, name="xn")
                    ssq = st_pool.tile([P, 1], f32, name="ssq")
                    nc.scalar.activation(xn[:], x_i[:], AF.Square,
                                         accum_out=ssq[:])
                    rms = st_pool.tile([P, 1], f32, name="rms")
                    nc.scalar.activation(rms[:], ssq[:], AF.Sqrt,
                                         scale=1.0 / C, bias=eps_t[:])
                    rs = st_pool.tile([P, 1], f32, name="rs")
                    nc.vector.reciprocal(rs[:], rms[:])
                    nc.vector.tensor_scalar(xn[:], x_i[:], rs[:], None, ALU.mult)
                    for g in range(2):   # two groups of 4 transposes
                        pt = psB.tile([P, 512], bf16, name="pt", tag="big")
                        for jj in range(4):
                            j = g * 4 + jj
                            nc.tensor.transpose(pt[:, jj * P:(jj + 1) * P],
                                                xn[:, j * P:(j + 1) * P],
                                                ident[:])
                        src = pt.rearrange("p (g n) -> p g n", n=P)
                        dst = xnt_v[:, g * 4:(g + 1) * 4, i * P:(i + 1) * P]
                        nc.vector.tensor_copy(dst, src)
                    # V projection for this token tile (needs only its own
                    # transposes; v columns arrive first on the gpsimd queue)
                    ps_v = psB.tile([P, N], f32, name="psv", tag="big")
                    for ch in range(2):
                        for j in range(CT):
                            nc.tensor.matmul(
                                ps_v[:, ch * 512:(ch + 1) * 512],
                                lhsT=xnt_v[:, j, i * P:(i + 1) * P],
                                rhs=wqkv_t[j][:, 2 * C + ch * 512:
                                              2 * C + (ch + 1) * 512],
                                start=(j == 0), stop=(j == CT - 1),
                                skip_group_check=True,
                            )
                    vdst = v_aug[i].rearrange("p (h e) -> p h e", e=D + 1)
                    vsrc = ps_v.rearrange("p (h d) -> p h d", d=D)
                    nc.vector.tensor_copy(vdst[:, :, 0:D], vsrc)
                    nc.vector.memset(vdst[:, :, D:D + 1], 1.0)


            # ---- helpers for the pair pipeline ----------------------------
            qk_q = {}
            qk_k = {}
            et_tiles = {}
            raw_tiles = {}
            pso_tiles = {}

            def qk_compute(p):
                """QKV projection + RoPE for pair p's q and k feature tiles."""
                for which in range(2):       # 0 = q, 1 = k
                    f = which * CT + p
                    ps = psB.tile([P, N], f32, name="psqk", tag="big")
                    for j in range(CT):
                        for ch in range(2):
                            nc.tensor.matmul(
                                ps[:, ch * 512:(ch + 1) * 512],
                                lhsT=wqkv_t[j][:, f * P:(f + 1) * P],
                                rhs=xnt_v[:, j, ch * 512:(ch + 1) * 512],
                                start=(j == 0), stop=(j == CT - 1),
                                skip_group_check=True,
                            )
                    qs = rt_pool.tile([P, N], bf16, name="qs", tag="qs")
                    nc.vector.tensor_copy(qs[:], ps[:])
                    t2 = rt_pool.tile([P, N], bf16, name="t2", tag="t2")
                    for b in (0, D):   # rotate-half cross terms per head block
                        nc.vector.tensor_mul(t2[b:b + 32, :],
                                             qs[b + 32:b + D, :],
                                             sinp[b + 32:b + D, :])
                        nc.vector.tensor_mul(t2[b + 32:b + D, :],
                                             qs[b:b + 32, :],
                                             sinp[b:b + 32, :])
                    dstp = qk_pool.tile([P, N], bf16, name=f"qk{f}",
                                        tag=f"qkf{which}")
                    nc.vector.tensor_mul(dstp[:], qs[:], cosp[:])
                    nc.vector.tensor_add(dstp[:], dstp[:], t2[:])
                    if which == 0:
                        qk_q[p] = dstp
                    else:
                        qk_k[p] = dstp

            def scores_step(i, k):
                """Packed 2-head score matmuls for pair i, step k.

                k = mt*2 + ch; psS cols 0:512 = head 2i, 512:1024 = head 2i+1,
                over query chunk ch. Raw scores are drained to SBUF fp16 by
                DVE (fast, keeps the PSUM ring off the exp critical path);
                exp runs from SBUF in [128, 2048] chunks per mt.
                """
                mt, ch = k // 2, k % 2
                ps = psB.tile([P, N], f32, name="psS", tag="big")
                qA = qk_q[i][0:D, ch * 512:(ch + 1) * 512]
                qB = qk_q[i][D:P, ch * 512:(ch + 1) * 512]
                kA = qk_k[i][0:D, mt * P:(mt + 1) * P]
                kB = qk_k[i][D:P, mt * P:(mt + 1) * P]
                nc.tensor.matmul(ps[:, 0:512], lhsT=kA, rhs=qA,
                                 start=True, stop=True)
                nc.tensor.matmul(ps[:, 512:1024], lhsT=kB, rhs=qB,
                                 start=True, stop=True)
                et = et_pool.tile([P, N], bf16, name="et", tag="et")
                nc.scalar.activation(et[:], ps[:], AF.Exp,
                                     scale=float(1.0 / np.sqrt(D)))
                et_tiles[(i, k)] = et

            def av_step(i, k):
                """A@V accumulation for pair i, step k (= mt*2 + ch)."""
                mt, ch = k // 2, k % 2
                if k == 0:
                    pso_tiles[i] = (
                        psO.tile([D + 1, N], f32, name="psoA", tag="pso"),
                        psO.tile([D + 1, N], f32, name="psoB", tag="pso"),
                    )
                psoA, psoB = pso_tiles[i]
                et = et_tiles.pop((i, k))
                vh = v_aug[mt].rearrange("p (h e) -> p h e", e=D + 1)
                start = (mt == 0)
                stop = (mt == NT - 1)
                nc.tensor.matmul(psoA[:, ch * 512:(ch + 1) * 512],
                                 lhsT=vh[:, 2 * i, :], rhs=et[:, 0:512],
                                 start=start, stop=stop, skip_group_check=True)
                nc.tensor.matmul(psoB[:, ch * 512:(ch + 1) * 512],
                                 lhsT=vh[:, 2 * i + 1, :], rhs=et[:, 512:1024],
                                 start=start, stop=stop, skip_group_check=True)

            def o_scale(i):
                """Divide accumulated o by the softmax denominator row.

                pso banks are released by two fast DVE copies into an SBUF
                staging tile; the reciprocal/broadcast/multiply chain then
                runs off the PE-critical path from the staged copy.
                partition_broadcast works in 512-col chunks only (the gpsimd
                ucode mishandles 4KB-per-partition broadcasts on HW), and
                the reciprocal must read a base-partition-0 SBUF tile.
                """
                psoA, psoB = pso_tiles.pop(i)
                stage = stg_pool.tile([P, 2 * N], bf16, name="stage",
                                      tag="stg")
                nc.vector.tensor_copy(stage[0:D + 1, 0:N], psoA[:])
                nc.vector.tensor_copy(stage[0:D + 1, N:2 * N], psoB[:])
                for hh in range(2):
                    for ch in range(2):
                        sl = slice(hh * N + ch * 512, hh * N + (ch + 1) * 512)
                        s_row = r_pool.tile([1, 512], f32, name="s_row",
                                            tag="sr")
                        nc.vector.tensor_copy(s_row[:], stage[D:D + 1, sl])
                        r_row = r_pool.tile([1, 512], f32, name="r_row",
                                            tag="rr")
                        nc.vector.reciprocal_approx_fast(r_row[:], s_row[:])
                        r_bf = r_pool.tile([1, 512], bf16, name="r_bf",
                                           tag="rb")
                        nc.vector.tensor_copy(r_bf[:], r_row[:])
                        rbs = rb_pool.tile([D, 512], bf16, name="rbs")
                        nc.gpsimd.partition_broadcast(rbs[:], r_bf[:])
                        nc.vector.tensor_tensor(
                            o_v[hh * D:(hh + 1) * D, i,
                                ch * 512:(ch + 1) * 512],
                            stage[0:D, sl], rbs[:], ALU.mult)

            # ---- fused pair pipeline --------------------------------------
            # iteration i: qk(i+1), scores(i,k) with av(i,k-2) interleaved,
            # then the av tail and the (staged) denominator scale
            qk_compute(0)
            for i in range(PAIRS):
                if i + 1 < PAIRS:
                    qk_compute(i + 1)
                for k in range(2 * NT):
                    scores_step(i, k)
                    if k >= 2:
                        av_step(i, k - 2)
                av_step(i, 2 * NT - 2)
                av_step(i, 2 * NT - 1)
                o_scale(i)

            # ---- proj + bias + out ---------------------------------------
            for nt in range(NT):
                ps = psB.tile([P, N], f32, name="psP", tag="big")
                for j in range(CT):
                    for ch in range(2):
                        nc.tensor.matmul(
                            ps[:, ch * 512:(ch + 1) * 512],
                            lhsT=o_v[:, j, nt * P:(nt + 1) * P],
                            rhs=wproj_v[:, j, ch * 512:(ch + 1) * 512],
                            start=(j == 0), stop=False,
                            skip_group_check=True,
                        )
                for ch in range(2):
                    nc.tensor.matmul(
                        ps[:, ch * 512:(ch + 1) * 512],
                        lhsT=ones_col[:],
                        rhs=bproj_sb[:, ch * 512:(ch + 1) * 512],
                        start=False, stop=True, skip_group_check=True,
                    )
                of = of_pool.tile([P, N], bf16, name="of")
                nc.vector.tensor_copy(of[:], ps[:])
                nc.sync.dma_start(out_ext[nt * P:(nt + 1) * P, :], of[:])

    nc.finalize()
    return nc


def _make_in_maps(x, scale, w_qkv, w_proj, b_proj):
    x = np.asarray(x, dtype=np.float32)
    scale = np.asarray(scale, dtype=np.float32)
    w_qkv = np.asarray(w_qkv, dtype=np.float32)
    w_proj = np.asarray(w_proj, dtype=np.float32)
    b_proj = np.asarray(b_proj, dtype=np.float32)

    # fold the RMSNorm scale into w_qkv (exact when scale == 1)
    w_eff = (scale[:, None] * w_qkv).astype(np.float32)
    # permute q/k head dims to rotate-half order [evens, odds]; scores are
    # invariant to a consistent permutation of the contraction dim
    perm = np.concatenate([np.arange(0, D, 2), np.arange(1, D, 2)])
    full_perm = (np.arange(H)[:, None] * D + perm[None, :]).reshape(-1)
    w_eff[:, 0:C] = w_eff[:, full_perm]
    w_eff[:, C:2 * C] = w_eff[:, C + full_perm]
    w_eff = w_eff.astype(BF16)
    wproj_b = w_proj.astype(BF16)
    bproj_b = b_proj.reshape(1, C).astype(BF16)
    cosp, sinp = _rope_tables_rh()
    cosp_b = cosp.astype(BF16)
    sinp_b = sinp.astype(BF16)

    in_maps = []
    for i in range(B):
        in_maps.append({
            "salt": np.zeros((1, _salt_len()), np.float32),
            "x": np.ascontiguousarray(x[i]).astype(BF16),
            "wqkv": w_eff,
            "wproj": wproj_b,
            "bproj": bproj_b,
            "cosp": cosp_b,
            "sinp": sinp_b,
        })
    return in_maps


def _run(inputs, trace=False):
    from concourse import bacc
    from concourse.bass_utils import run_bass_kernel_spmd

    nc = build(bacc.Bacc())
    in_maps = _make_in_maps(**inputs)
    res = run_bass_kernel_spmd(nc, in_maps, list(range(B)), trace=trace)
    out = np.stack([np.asarray(res.results[i]["out"], dtype=np.float32)
                    for i in range(B)], axis=0)
    return out, res


def kernel(x, scale, w_qkv, w_proj, b_proj):
    out, _ = _run(dict(x=x, scale=scale, w_qkv=w_qkv, w_proj=w_proj,
                       b_proj=b_proj))
    return out


# revision 31
# speedup vs baseline: 1.0705x; 1.0705x over previous
"""Fused attention block (RMSNorm -> QKV -> 2D RoPE -> SDPA -> proj) on 8
Trainium2 NeuronCores, data-parallel over the batch dimension (B=8, one batch
element per core; no collectives).

Pipelined per-head-pair schedule: iteration i computes the QKV feature tiles
for pair i+1, the (row-tiled, 2-head-packed) score matmuls + softmax exp for
pair i, and the A@V matmuls for pair i-1, so the PE array and the scalar
(activation) engine stay busy concurrently.

Self-contained: hardcodes shapes B=8, N=1024, C=1024, H=16, D=64.
"""

import hashlib
import numpy as np
import ml_dtypes


def _salt_len():
    """Source-dependent dummy-input length: the remote AOT compile cache
    keys on the HLO signature and misses kernel-program changes, so vary
    the signature whenever this file changes."""
    try:
        with open(__file__, 'rb') as f:
            h = hashlib.sha256(f.read()).digest()
        return int.from_bytes(h[:2], 'little') + 1
    except OSError:
        return 1

B = 8
N = 1024
C = 1024
H = 16
D = 64
GRID = 32
EPS = 1e-6
P = 128
NT = N // P      # 8 token tiles
CT = C // P      # 8 channel tiles
PAIRS = H // 2   # 8 head pairs

BF16 = ml_dtypes.bfloat16


def _rope_tables():
    """Feature-major 2D-RoPE tables [N, 32] (cos/sin per rotation pair)."""
    quarter = D // 4
    freqs = 1.0 / (10000.0 ** (np.arange(quarter, dtype=np.float64) / quarter))
    ys, xs = np.meshgrid(np.arange(GRID, dtype=np.float64),
                         np.arange(GRID, dtype=np.float64), indexing='ij')
    ang_x = xs.reshape(-1)[:, None] * freqs[None, :]      # [N, 16]
    ang_y = ys.reshape(-1)[:, None] * freqs[None, :]      # [N, 16]
    angles = np.concatenate([ang_x, ang_y], axis=-1)      # [N, 32]
    return np.cos(angles), np.sin(angles)


def _rope_tables_rh():
    """Rotate-half-layout tables [128, N] for the permuted q/k dim order.

    Head dims are permuted to [evens, odds]; within each 64-dim head block:
      out[0:32]  = q[0:32]*cos - q[32:64]*sin
      out[32:64] = q[0:32]*sin + q[32:64]*cos
    so cosp = [cos, cos] and sinp = [-sin, +sin] per block, duplicated for
    the two heads sharing a feature tile.
    """
    cos, sin = _rope_tables()                             # [N, 32]
    blk_cos = np.concatenate([cos.T, cos.T], axis=0)      # [64, N]
    # sin sign keyed by SOURCE half (cross-term reads the opposite half, and
    # walrus requires both SBUF inputs of tensor_tensor to share a base
    # partition): reading odds (rows 32:64) contributes -sin, evens +sin.
    blk_sin = np.concatenate([sin.T, -sin.T], axis=0)     # [64, N]
    cosp = np.concatenate([blk_cos, blk_cos], axis=0)     # [128, N]
    sinp = np.concatenate([blk_sin, blk_sin], axis=0)
    return cosp.astype(np.float32), sinp.astype(np.float32)


def build(nc):
    import concourse.mybir as mybir
    import concourse.tile as tile
    from concourse.masks import make_identity

    f32 = mybir.dt.float32
    f16 = mybir.dt.float16
    bf16 = mybir.dt.bfloat16
    AF = mybir.ActivationFunctionType
    ALU = mybir.AluOpType

    x_ext = nc.declare_dram_parameter("x", [N, C], bf16, isOutput=False)
    wqkv_ext = nc.declare_dram_parameter("wqkv", [C, 3 * C], bf16, isOutput=False)
    wproj_ext = nc.declare_dram_parameter("wproj", [C, C], bf16, isOutput=False)
    bproj_ext = nc.declare_dram_parameter("bproj", [1, C], bf16, isOutput=False)
    cosp_ext = nc.declare_dram_parameter("cosp", [P, N], bf16, isOutput=False)
    sinp_ext = nc.declare_dram_parameter("sinp", [P, N], bf16, isOutput=False)
    out_ext = nc.declare_dram_parameter("out", [N, C], bf16, isOutput=True)
    nc.declare_dram_parameter("salt", [1, _salt_len()], f32, isOutput=False)

    from contextlib import ExitStack

    with tile.TileContext(nc) as tc:
        with ExitStack() as stack:
            ep = stack.enter_context
            consts = ep(tc.tile_pool(name="consts", bufs=1))
            w_pool = ep(tc.tile_pool(name="weights", bufs=1))
            tabs = ep(tc.tile_pool(name="tabs", bufs=1))
            xnt_pool = ep(tc.tile_pool(name="xnt", bufs=1))
            v_pool = ep(tc.tile_pool(name="vaug", bufs=1))
            o_pool = ep(tc.tile_pool(name="ofm", bufs=1))
            psB = ep(tc.tile_pool(name="psbig", bufs=2, space="PSUM"))
            psO = ep(tc.tile_pool(name="pso", bufs=2, space="PSUM"))
            qk_pool = ep(tc.tile_pool(name="qk", bufs=2))
            rt_pool = ep(tc.tile_pool(name="rope", bufs=2))
            et_pool = ep(tc.tile_pool(name="et", bufs=16))
            stg_pool = ep(tc.tile_pool(name="stg", bufs=1))
            r_pool = ep(tc.tile_pool(name="rstat", bufs=1))
            rb_pool = ep(tc.tile_pool(name="rbc", bufs=1))
            of_pool = ep(tc.tile_pool(name="outsb", bufs=2))
            ident = consts.tile([P, P], bf16, name="ident")
            make_identity(nc, ident[:])
            ones_col = consts.tile([1, P], bf16, name="ones_col")
            nc.vector.memset(ones_col[:], 1.0)
            eps_t = consts.tile([P, 1], f32, name="eps_t")
            nc.vector.memset(eps_t[:], EPS)
            bproj_sb = consts.tile([1, C], bf16, name="bproj_sb")
            nc.scalar.dma_start(bproj_sb[:], bproj_ext[:])
            cosp = tabs.tile([P, N], bf16, name="cosp")
            sinp = tabs.tile([P, N], bf16, name="sinp")
            nc.scalar.dma_start(cosp[:], cosp_ext[:])
            nc.scalar.dma_start(sinp[:], sinp_ext[:])

            # weights: wqkv per channel tile j; wproj one big tile
            wqkv_t = [w_pool.tile([P, 3 * C], bf16, name=f"wq{j}", tag=f"wq{j}")
                      for j in range(CT)]
            wproj_b = w_pool.tile([P, CT * C], bf16, name="wproj_b")
            # DMA order: q cols, k cols, v cols (matches first-use order),
            # wproj last on a separate queue.
            wproj_v = wproj_b.rearrange("p (j c) -> p j c", c=C)

            # xnt_big[:, j*N + n] = xn[n, j*128 + p] (normalized x, transposed)
            xnt_big = xnt_pool.tile([P, CT * N], bf16, name="xnt_big")
            xnt_v = xnt_big.rearrange("p (j n) -> p j n", n=N)
            # v token-major with ones column: [128, 16*65] per token tile
            v_aug = [v_pool.tile([P, H * (D + 1)], bf16, name=f"vaug{i}",
                                 tag=f"vaug{i}") for i in range(NT)]
            # attention output, feature-major: pair f at cols f*N..(f+1)*N
            o_big = o_pool.tile([P, PAIRS * N], bf16, name="o_big")
            o_v = o_big.rearrange("p (f n) -> p f n", n=N)

            # ---- phase A: norm + transpose --------------------------------
            with ExitStack() as astack:
                x_pool = astack.enter_context(tc.tile_pool(name="xin", bufs=4))
                st_pool = astack.enter_context(tc.tile_pool(name="stats", bufs=2))
                xn_pool = astack.enter_context(tc.tile_pool(name="xn", bufs=2))
                x_tiles = []
                for i in range(NT):
                    x_i = x_pool.tile([P, C], bf16, name="x_i")
                    x_tiles.append(x_i)
                for i in (0, 2, 1, 3, 5, 7):
                    eng = nc.sync if i % 2 == 0 else nc.scalar
                    eng.dma_start(x_tiles[i][:], x_ext[i * P:(i + 1) * P, :])
                # weight DMAs issue behind the ungated x tiles: q on sync
                # (scalar must stay free for the norm ACTs), then the
                # ring-gated x4/x6; k/v/wproj on gpsimd
                for j in range(CT):
                    nc.sync.dma_start(wqkv_t[j][:, 0:C],
                                      wqkv_ext[j * P:(j + 1) * P, 0:C])
                for i in (4, 6):
                    nc.sync.dma_start(x_tiles[i][:],
                                      x_ext[i * P:(i + 1) * P, :])
                for j in range(CT):
                    nc.gpsimd.dma_start(wqkv_t[j][:, 2 * C:],
                                        wqkv_ext[j * P:(j + 1) * P, 2 * C:])
                for j in range(CT):
                    nc.gpsimd.dma_start(wqkv_t[j][:, C:2 * C],
                                        wqkv_ext[j * P:(j + 1) * P, C:2 * C])
                for j in range(CT):
                    nc.gpsimd.dma_start(wproj_v[:, j, :],
                                        wproj_ext[j * P:(j + 1) * P, :])
                for i in range(NT):
                    x_i = x_tiles[i]
                    xn = xn_pool.tile([P, C], bf16, name="xn")
                    ssq = st_pool.tile([P, 1], f32, name="ssq")
                    nc.scalar.activation(xn[:], x_i[:], AF.Square,
                                         accum_out=ssq[:])
                    rms = st_pool.tile([P, 1], f32, name="rms")
                    nc.scalar.activation(rms[:], ssq[:], AF.Sqrt,
                                         scale=1.0 / C, bias=eps_t[:])
                    rs = st_pool.tile([P, 1], f32, name="rs")
                    nc.vector.reciprocal(rs[:], rms[:])
                    nc.vector.tensor_scalar(xn[:], x_i[:], rs[:], None, ALU.mult)
                    for g in range(2):   # two groups of 4 transposes
                        pt = psB.tile([P, 512], bf16, name="pt", tag="big")
                        for jj in range(4):
                            j = g * 4 + jj
                            nc.tensor.transpose(pt[:, jj * P:(jj + 1) * P],
                                                xn[:, j * P:(j + 1) * P],
                                                ident[:])
                        src = pt.rearrange("p (g n) -> p g n", n=P)
                        dst = xnt_v[:, g * 4:(g + 1) * 4, i * P:(i + 1) * P]
                        nc.vector.tensor_copy(dst, src)
                    # V projection for this token tile (needs only its own
                    # transposes; v columns arrive first on the gpsimd queue)
                    ps_v = psB.tile([P, N], f32, name="psv", tag="big")
                    for ch in range(2):
                        for j in range(CT):
                            nc.tensor.matmul(
                                ps_v[:, ch * 512:(ch + 1) * 512],
                                lhsT=xnt_v[:, j, i * P:(i + 1) * P],
                                rhs=wqkv_t[j][:, 2 * C + ch * 512:
                                              2 * C + (ch + 1) * 512],
                                start=(j == 0), stop=(j == CT - 1),
                                skip_group_check=True,
                            )
                    vdst = v_aug[i].rearrange("p (h e) -> p h e", e=D + 1)
                    vsrc = ps_v.rearrange("p (h d) -> p h d", d=D)
                    nc.vector.tensor_copy(vdst[:, :, 0:D], vsrc)
                    nc.vector.memset(vdst[:, :, D:D + 1], 1.0)


            # ---- helpers for the pair pipeline ----------------------------
            qk_q = {}
            qk_k = {}
            et_tiles = {}
            raw_tiles = {}
            pso_tiles = {}

            def qk_compute(p):
                """QKV projection + RoPE for pair p's q and k feature tiles."""
                for which in range(2):       # 0 = q, 1 = k
                    f = which * CT + p
                    ps = psB.tile([P, N], f32, name="psqk", tag="big")
                    for j in range(CT):
                        for ch in range(2):
                            nc.tensor.matmul(
                                ps[:, ch * 512:(ch + 1) * 512],
                                lhsT=wqkv_t[j][:, f * P:(f + 1) * P],
                                rhs=xnt_v[:, j, ch * 512:(ch + 1) * 512],
                                start=(j == 0), stop=(j == CT - 1),
                                skip_group_check=True,
                            )
                    qs = rt_pool.tile([P, N], bf16, name="qs", tag="qs")
                    nc.vector.tensor_copy(qs[:], ps[:])
                    t2 = rt_pool.tile([P, N], bf16, name="t2", tag="t2")
                    for b in (0, D):   # rotate-half cross terms per head block
                        nc.vector.tensor_mul(t2[b:b + 32, :],
                                             qs[b + 32:b + D, :],
                                             sinp[b + 32:b + D, :])
                        nc.vector.tensor_mul(t2[b + 32:b + D, :],
                                             qs[b:b + 32, :],
                                             sinp[b:b + 32, :])
                    dstp = qk_pool.tile([P, N], bf16, name=f"qk{f}",
                                        tag=f"qkf{which}")
                    nc.vector.tensor_mul(dstp[:], qs[:], cosp[:])
                    nc.vector.tensor_add(dstp[:], dstp[:], t2[:])
                    if which == 0:
                        qk_q[p] = dstp
                    else:
                        qk_k[p] = dstp

            def scores_step(i, k):
                """Packed 2-head score matmuls for pair i, step k.

                k = mt*2 + ch; psS cols 0:512 = head 2i, 512:1024 = head 2i+1,
                over query chunk ch. Raw scores are drained to SBUF fp16 by
                DVE (fast, keeps the PSUM ring off the exp critical path);
                exp runs from SBUF in [128, 2048] chunks per mt.
                """
                mt, ch = k // 2, k % 2
                ps = psB.tile([P, N], f32, name="psS", tag="big")
                qA = qk_q[i][0:D, ch * 512:(ch + 1) * 512]
                qB = qk_q[i][D:P, ch * 512:(ch + 1) * 512]
                kA = qk_k[i][0:D, mt * P:(mt + 1) * P]
                kB = qk_k[i][D:P, mt * P:(mt + 1) * P]
                nc.tensor.matmul(ps[:, 0:512], lhsT=kA, rhs=qA,
                                 start=True, stop=True)
                nc.tensor.matmul(ps[:, 512:1024], lhsT=kB, rhs=qB,
                                 start=True, stop=True)
                et = et_pool.tile([P, N], bf16, name="et", tag="et")
                nc.scalar.activation(et[:], ps[:], AF.Exp,
                                     scale=float(1.0 / np.sqrt(D)))
                et_tiles[(i, k)] = et

            def av_step(i, k):
                """A@V accumulation for pair i, step k (= mt*2 + ch)."""
                mt, ch = k // 2, k % 2
                if k == 0:
                    pso_tiles[i] = (
                        psO.tile([D + 1, N], f32, name="psoA", tag="pso"),
                        psO.tile([D + 1, N], f32, name="psoB", tag="pso"),
                    )
                psoA, psoB = pso_tiles[i]
                et = et_tiles.pop((i, k))
                vh = v_aug[mt].rearrange("p (h e) -> p h e", e=D + 1)
                start = (mt == 0)
                stop = (mt == NT - 1)
                nc.tensor.matmul(psoA[:, ch * 512:(ch + 1) * 512],
                                 lhsT=vh[:, 2 * i, :], rhs=et[:, 0:512],
                                 start=start, stop=stop, skip_group_check=True)
                nc.tensor.matmul(psoB[:, ch * 512:(ch + 1) * 512],
                                 lhsT=vh[:, 2 * i + 1, :], rhs=et[:, 512:1024],
                                 start=start, stop=stop, skip_group_check=True)

            def o_scale(i):
                """Divide accumulated o by the softmax denominator row.

                pso banks are released by two fast DVE copies into an SBUF
                staging tile; the reciprocal/broadcast/multiply chain then
                runs off the PE-critical path from the staged copy.
                partition_broadcast works in 512-col chunks only (the gpsimd
                ucode mishandles 4KB-per-partition broadcasts on HW), and
                the reciprocal must read a base-partition-0 SBUF tile.
                """
                psoA, psoB = pso_tiles.pop(i)
                stage = stg_pool.tile([P, 2 * N], bf16, name="stage",
                                      tag="stg")
                nc.vector.tensor_copy(stage[0:D + 1, 0:N], psoA[:])
                nc.vector.tensor_copy(stage[0:D + 1, N:2 * N], psoB[:])
                for hh in range(2):
                    for ch in range(2):
                        sl = slice(hh * N + ch * 512, hh * N + (ch + 1) * 512)
                        s_row = r_pool.tile([1, 512], f32, name="s_row",
                                            tag="sr")
                        nc.vector.tensor_copy(s_row[:], stage[D:D + 1, sl])
                        r_row = r_pool.tile([1, 512], f32, name="r_row",
                                            tag="rr")
                        nc.vector.reciprocal_approx_fast(r_row[:], s_row[:])
                        r_bf = r_pool.tile([1, 512], bf16, name="r_bf",
                                           tag="rb")
                        nc.vector.tensor_copy(r_bf[:], r_row[:])
                        rbs = rb_pool.tile([D, 512], bf16, name="rbs")
                        nc.gpsimd.partition_broadcast(rbs[:], r_bf[:])
                        nc.vector.tensor_tensor(
                            o_v[hh * D:(hh + 1) * D, i,
                                ch * 512:(ch + 1) * 512],
                            stage[0:D, sl], rbs[:], ALU.mult)

            # ---- fused pair pipeline --------------------------------------
            # iteration i: qk(i+1), scores(i,k) with av(i,k-2) interleaved,
            # then the av tail and the (staged) denominator scale
            qk_compute(0)
            for i in range(PAIRS):
                if i + 1 < PAIRS:
                    qk_compute(i + 1)
                for k in range(2 * NT):
                    scores_step(i, k)
                    if i > 0:
                        av_step(i - 1, k)
                if i > 0:
                    o_scale(i - 1)
            for k in range(2 * NT):
                av_step(PAIRS - 1, k)
            o_scale(PAIRS - 1)

            # ---- proj + bias + out ---------------------------------------
            for nt in range(NT):
                ps = psB.tile([P, N], f32, name="psP", tag="big")
                for j in range(CT):
                    for ch in range(2):
                        nc.tensor.matmul(
                            ps[:, ch * 512:(ch + 1) * 512],
                            lhsT=o_v[:, j, nt * P:(nt + 1) * P],
                            rhs=wproj_v[:, j, ch * 512:(ch + 1) * 512],
                            start=(j == 0), stop=False,
                            skip_group_check=True,
                        )
                for ch in range(2):
                    nc.tensor.matmul(
                        ps[:, ch * 512:(ch + 1) * 512],
                        lhsT=ones_col[:],
                        rhs=bproj_sb[:, ch * 512:(ch + 1) * 512],
                        start=False, stop=True, skip_group_check=True,
                    )
                of = of_pool.tile([P, N], bf16, name="of")
                nc.vector.tensor_copy(of[:], ps[:])
                nc.sync.dma_start(out_ext[nt * P:(nt + 1) * P, :], of[:])

    nc.finalize()
    return nc


def _make_in_maps(x, scale, w_qkv, w_proj, b_proj):
    x = np.asarray(x, dtype=np.float32)
    scale = np.asarray(scale, dtype=np.float32)
    w_qkv = np.asarray(w_qkv, dtype=np.float32)
    w_proj = np.asarray(w_proj, dtype=np.float32)
    b_proj = np.asarray(b_proj, dtype=np.float32)

    # fold the RMSNorm scale into w_qkv (exact when scale == 1)
    w_eff = (scale[:, None] * w_qkv).astype(np.float32)
    # permute q/k head dims to rotate-half order [evens, odds]; scores are
    # invariant to a consistent permutation of the contraction dim
    perm = np.concatenate([np.arange(0, D, 2), np.arange(1, D, 2)])
    full_perm = (np.arange(H)[:, None] * D + perm[None, :]).reshape(-1)
    w_eff[:, 0:C] = w_eff[:, full_perm]
    w_eff[:, C:2 * C] = w_eff[:, C + full_perm]
    w_eff = w_eff.astype(BF16)
    wproj_b = w_proj.astype(BF16)
    bproj_b = b_proj.reshape(1, C).astype(BF16)
    cosp, sinp = _rope_tables_rh()
    cosp_b = cosp.astype(BF16)
    sinp_b = sinp.astype(BF16)

    in_maps = []
    for i in range(B):
        in_maps.append({
            "salt": np.zeros((1, _salt_len()), np.float32),
            "x": np.ascontiguousarray(x[i]).astype(BF16),
            "wqkv": w_eff,
            "wproj": wproj_b,
            "bproj": bproj_b,
            "cosp": cosp_b,
            "sinp": sinp_b,
        })
    return in_maps


def _run(inputs, trace=False):
    from concourse import bacc
    from concourse.bass_utils import run_bass_kernel_spmd

    nc = build(bacc.Bacc())
    in_maps = _make_in_maps(**inputs)
    res = run_bass_kernel_spmd(nc, in_maps, list(range(B)), trace=trace)
    out = np.stack([np.asarray(res.results[i]["out"], dtype=np.float32)
                    for i in range(B)], axis=0)
    return out, res


def kernel(x, scale, w_qkv, w_proj, b_proj):
    out, _ = _run(dict(x=x, scale=scale, w_qkv=w_qkv, w_proj=w_proj,
                       b_proj=b_proj))
    return out
